# revision 6
# baseline (speedup 1.0000x reference)
"""MaxMarginLoss Trainium2 kernel (8 NeuronCores, vocab-sharded).

Math (reference):
    out_norm = l2norm(preds^T over D)            [B,S,D]
    voc_norm = l2norm(emb over D)                [V,D]
    tgt      = emb[target]                       [B,S,D]
    d        = out_norm@voc_norm.T - tgt@voc_norm.T
    jmax     = argmax_v d
    loss     = mean_masked(relu(g + cos[jmax] - cos[target]))

Key identity: d = (out_norm - tgt) @ voc_norm.T  -> ONE matmul instead of two.
Per-row positive scaling doesn't change argmax, so each device computes
    slab[s,v] = (preds[s] - n_s*tgt[s]) . voc_norm[v]     ( = n_s * d[s,v] )
which avoids any division on device.  Per core the slab is [2048, 4000]
(vocab shard); argmax per 128-row tile via DVE max8+max_index, winning emb
rows gathered by indirect DMA, then raw dots s1=preds.emb[jmax],
s2=|emb[jmax]|^2 let the host finish:
    cos[jmax]   = s1 / (sqrt(s2) * n_s)
    cos[target] = (preds.tgt) / (|tgt| * n_s)        (host, input-only)
Host combines the 8 per-core (max, argmax) candidates (first-max tie-break
matches jnp.argmax since shards are vocab-ordered) and takes the masked mean.
Host prep also supplies the row norms n_s and 1/|emb_v| (input statistics);
the heavy compute (matmul, argmax scans, gathers, argmax-dependent dots)
all runs on device.
"""

import os
import sys

import numpy as np

for _p in ("/opt/trn_rl_repo", "/root/.axon_site/_ro/trn_rl_repo"):
    if os.path.isdir(_p) and _p not in sys.path:
        sys.path.insert(0, _p)

import concourse.bass as bass
import concourse.bacc as bacc_mod
import concourse.mybir as mybir
from concourse.tile import TileContext

P = 128
B, S, D, V = 4, 512, 512, 32000
BS = B * S                  # 2048 rows
NCORES = 8
VS = V // NCORES            # 4000 vocab rows per core
KC = D // P                 # 4 contraction chunks
NT = BS // P                # 16 row tiles
SC = BS // 512              # 4 512-col chunks of the row dim
VCH = [512] * 7 + [416]     # vocab chunks per core (sum 4000)
VOFF = [sum(VCH[:i]) for i in range(len(VCH))]
GAMMA = 0.5

F32 = mybir.dt.float32
U32 = mybir.dt.uint32
MM_DT = mybir.dt.bfloat16   # matmul input dtype (fp32 accumulate in PSUM)

_CACHED = {}


def build_nc():
    nc = bacc_mod.Bacc()

    predsT = nc.declare_dram_parameter("predsT", [D, BS], F32, isOutput=False)
    tgtT = nc.declare_dram_parameter("tgtT", [D, BS], F32, isOutput=False)
    predsN = nc.declare_dram_parameter("predsN", [BS, D], F32, isOutput=False)
    embT = nc.declare_dram_parameter("embT", [D, VS], F32, isOutput=False)
    embN = nc.declare_dram_parameter("embN", [VS, D], F32, isOutput=False)
    invV = nc.declare_dram_parameter("invV", [1, VS], F32, isOutput=False)
    nrow = nc.declare_dram_parameter("nrow", [1, BS], F32, isOutput=False)

    o_maxv = nc.declare_dram_parameter("o_maxv", [P, NT], F32, isOutput=True)
    o_idx = nc.declare_dram_parameter("o_idx", [P, NT], U32, isOutput=True)
    o_s1 = nc.declare_dram_parameter("o_s1", [P, NT], F32, isOutput=True)
    o_s2 = nc.declare_dram_parameter("o_s2", [P, NT], F32, isOutput=True)

    with TileContext(nc) as tc:
        with (
            tc.tile_pool(name="const", bufs=1) as cpool,
            tc.tile_pool(name="pieces", bufs=4) as pc,
            tc.tile_pool(name="bcvp", bufs=2) as bcvp,
            tc.tile_pool(name="gp", bufs=2) as gp,
            tc.tile_pool(name="slabp", bufs=3) as slabp,
            tc.tile_pool(name="smallp", bufs=4) as smallp,
            tc.tile_pool(name="junkp", bufs=2) as junkp,
            tc.tile_pool(name="psp", bufs=8, space="PSUM") as psp,
        ):
            # broadcast of the row norms across partitions (DMA from input)
            bc_n = cpool.tile([P, BS], F32, tag="bc_n")
            nc.sync.dma_start(bc_n, nrow[0:1, :].to_broadcast([P, BS]))

            # persistent matmul operands, piece-granular for early readiness
            eT = [[cpool.tile([P, 512], MM_DT, tag=f"eT{k}_{sc}", name=f"eT{k}_{sc}")
                   for sc in range(SC)] for k in range(KC)]
            vocnT = [[cpool.tile([P, VCH[vc]], MM_DT, tag=f"vocnT{k}_{vc}",
                                 name=f"vocnT{k}_{vc}")
                      for vc in range(len(VCH))] for k in range(KC)]

            # outputs accumulate in SBUF, DMA'd once at the end
            maxv_sb = cpool.tile([P, NT], F32, tag="maxv_sb")
            idx_sb = cpool.tile([P, NT], U32, tag="idx_sb")
            s1_sb = cpool.tile([P, NT], F32, tag="s1_sb")
            s2_sb = cpool.tile([P, NT], F32, tag="s2_sb")

            # ---- Stage A2: eT[k][sc] = predsT - tgtT * bc_n  (bf16) ----------
            for sc in range(SC):
                for k in range(KC):
                    cs = slice(sc * 512, (sc + 1) * 512)
                    pA = pc.tile([P, 512], F32, tag="pA")
                    nc.sync.dma_start(pA, predsT[k * P:(k + 1) * P, cs])
                    tA = pc.tile([P, 512], F32, tag="tA")
                    nc.sync.dma_start(tA, tgtT[k * P:(k + 1) * P, cs])
                    t1 = gp.tile([P, 512], F32, tag="t1")
                    nc.gpsimd.tensor_mul(t1, tA, bc_n[:, cs])
                    nc.gpsimd.tensor_sub(eT[k][sc], pA, t1)

            # ---- Stage A3: vocnT[k][vc] = embT * bcast(invV)  (bf16) ---------
            for vc in range(len(VCH)):
                w, off = VCH[vc], VOFF[vc]
                bcv = bcvp.tile([P, 512], F32, tag="bcv")
                nc.sync.dma_start(
                    bcv[:, :w], invV[0:1, off:off + w].to_broadcast([P, w]))
                for k in range(KC):
                    eM = pc.tile([P, 512], F32, tag="eM")
                    nc.sync.dma_start(eM[:, :w], embT[k * P:(k + 1) * P, off:off + w])
                    nc.gpsimd.tensor_mul(vocnT[k][vc], eM[:, :w], bcv[:, :w])

            # ---- Stage B: per row-tile matmul + argmax + gather + dots -------
            for t in range(NT):
                ts = slice(t * P, (t + 1) * P)
                sc_t, r_t = t // 4, (t % 4) * P
                slab = slabp.tile([P, VS], F32, tag="slab")
                for vc in range(len(VCH)):
                    w, off = VCH[vc], VOFF[vc]
                    ps = psp.tile([P, 512], F32, tag="ps")
                    for k in range(KC):
                        nc.tensor.matmul(
                            ps[:, :w],
                            lhsT=eT[k][sc_t][:, r_t:r_t + P],
                            rhs=vocnT[k][vc],
                            start=(k == 0),
                            stop=(k == KC - 1),
                        )
                    nc.scalar.copy(slab[:, off:off + w], ps[:, :w])

                m8 = smallp.tile([P, 8], F32, tag="m8")
                i8 = smallp.tile([P, 8], U32, tag="i8")
                nc.vector.max(out=m8, in_=slab)
                nc.vector.max_index(out=i8, in_max=m8, in_values=slab)
                nc.vector.tensor_copy(maxv_sb[:, t:t + 1], m8[:, 0:1])
                nc.vector.tensor_copy(idx_sb[:, t:t + 1], i8[:, 0:1])

                gath = pc.tile([P, D], F32, tag="gath")
                nc.gpsimd.indirect_dma_start(
                    out=gath,
                    out_offset=None,
                    in_=embN[:],
                    in_offset=bass.IndirectOffsetOnAxis(ap=i8[:, 0:1], axis=0),
                )

                pN = pc.tile([P, D], F32, tag="pN")
                nc.sync.dma_start(pN, predsN[ts, :])

                m1 = gp.tile([P, D], F32, tag="m1")
                nc.gpsimd.tensor_mul(m1, gath, pN)
                j1 = junkp.tile([P, D], F32, tag="j1")
                nc.scalar.activation(
                    j1, m1, mybir.ActivationFunctionType.Copy,
                    accum_out=s1_sb[:, t:t + 1],
                )
                j2 = junkp.tile([P, D], F32, tag="j1")
                nc.scalar.activation(
                    j2, gath, mybir.ActivationFunctionType.Square,
                    accum_out=s2_sb[:, t:t + 1],
                )

            nc.sync.dma_start(o_maxv[:], maxv_sb)
            nc.sync.dma_start(o_idx[:], idx_sb)
            nc.sync.dma_start(o_s1[:], s1_sb)
            nc.sync.dma_start(o_s2[:], s2_sb)

    return nc


def get_nc():
    if "nc" not in _CACHED:
        _CACHED["nc"] = build_nc()
    return _CACHED["nc"]


def make_in_maps(preds, emb_weight, target):
    """Host-side input prep: layouts, shards, target-row gather, norms."""
    preds = np.ascontiguousarray(np.asarray(preds, dtype=np.float32))      # [B,D,S]
    emb = np.ascontiguousarray(np.asarray(emb_weight, dtype=np.float32))   # [V,D]
    tgt_idx = np.asarray(target).astype(np.int64).reshape(-1)              # [BS]

    # loss row index j = b*S + s
    predsT = np.ascontiguousarray(preds.transpose(1, 0, 2).reshape(D, BS))
    predsN = np.ascontiguousarray(preds.transpose(0, 2, 1).reshape(BS, D))
    tgtN = np.ascontiguousarray(emb[tgt_idx])                              # [BS, D]
    tgtT = np.ascontiguousarray(tgtN.T)                                    # [D, BS]
    invV_full = (1.0 / np.sqrt((emb ** 2).sum(axis=1))).astype(np.float32)
    nrow = np.sqrt((predsN ** 2).sum(axis=1)).astype(np.float32)

    in_maps = []
    for c in range(NCORES):
        sl = slice(c * VS, (c + 1) * VS)
        emb_shard = emb[sl]
        in_maps.append({
            "predsT": predsT,
            "tgtT": tgtT,
            "predsN": predsN,
            "embT": np.ascontiguousarray(emb_shard.T),
            "embN": np.ascontiguousarray(emb_shard),
            "invV": np.ascontiguousarray(invV_full[sl].reshape(1, VS)),
            "nrow": np.ascontiguousarray(nrow.reshape(1, BS)),
        })
    return in_maps


def combine(results, preds, emb_weight, target, pad_id):
    """Host-side unshard: pick global argmax winner per row, finish the loss."""
    preds = np.asarray(preds, dtype=np.float32)
    emb = np.asarray(emb_weight, dtype=np.float32)
    tgt_idx = np.asarray(target).astype(np.int64).reshape(-1)

    def flat(a):  # [P, NT] laid out so row index j = t*128 + p
        return np.asarray(a).T.reshape(-1)

    maxv = np.stack([flat(r["o_maxv"]) for r in results])          # [8, BS]
    s1 = np.stack([flat(r["o_s1"]) for r in results])
    s2 = np.stack([flat(r["o_s2"]) for r in results])

    predsN = preds.transpose(0, 2, 1).reshape(BS, D)
    n_s = np.sqrt((predsN ** 2).sum(axis=1))
    tgtN = emb[tgt_idx]
    s3 = (predsN * tgtN).sum(axis=1)
    s4 = (tgtN * tgtN).sum(axis=1)

    # winner core per row; np.argmax picks the first (lowest shard => lowest
    # global index) on exact ties, matching jnp.argmax first-occurrence.
    win = np.argmax(maxv, axis=0)                                  # [BS]
    rows = np.arange(BS)
    s1w = s1[win, rows]
    s2w = s2[win, rows]

    max_cos = s1w / (np.sqrt(s2w) * n_s)
    cos_tgt = s3 / (np.sqrt(s4) * n_s)
    diff = np.maximum(np.float32(GAMMA) + max_cos - cos_tgt, 0.0).astype(np.float32)
    mask = tgt_idx != int(np.asarray(pad_id))
    denom = np.float32(mask.sum())
    loss = np.float32(np.where(mask, diff, np.float32(0.0)).sum() / denom)
    return np.asarray(loss, dtype=np.float32)


def run_cores(in_maps, trace=False):
    from concourse.bass_utils import run_bass_kernel_spmd
    nc = get_nc()
    if not nc.is_finalized():
        nc.finalize()
    return run_bass_kernel_spmd(nc, in_maps, list(range(NCORES)), trace=trace)


def kernel(preds, emb_weight, target, pad_id):
    in_maps = make_in_maps(preds, emb_weight, target)
    res = run_cores(in_maps, trace=False)
    return combine(res.results, preds, emb_weight, target, pad_id)


# revision 7
# speedup vs baseline: 1.1544x; 1.1544x over previous
"""MaxMarginLoss Trainium2 kernel (8 NeuronCores, vocab-sharded).

Math (reference):
    out_norm = l2norm(preds^T over D)            [B,S,D]
    voc_norm = l2norm(emb over D)                [V,D]
    tgt      = emb[target]                       [B,S,D]
    d        = out_norm@voc_norm.T - tgt@voc_norm.T
    jmax     = argmax_v d
    loss     = mean_masked(relu(g + cos[jmax] - cos[target]))

Key identity: d = (out_norm - tgt) @ voc_norm.T  -> ONE matmul instead of two.
Per-row positive scaling doesn't change argmax, so each device computes
    slab[s,v] = (preds[s] - n_s*tgt[s]) . voc_norm[v]     ( = n_s * d[s,v] )
which avoids any division on device.  Per core the slab is [2048, 4000]
(vocab shard); argmax per 128-row tile via DVE max8+max_index, winning emb
rows gathered by indirect DMA, then raw dots s1=preds.emb[jmax],
s2=|emb[jmax]|^2 let the host finish:
    cos[jmax]   = s1 / (sqrt(s2) * n_s)
    cos[target] = (preds.tgt) / (|tgt| * n_s)        (host, input-only)
Host combines the 8 per-core (max, argmax) candidates (first-max tie-break
matches jnp.argmax since shards are vocab-ordered) and takes the masked mean.
Host prep also supplies the row norms n_s and 1/|emb_v| (input statistics);
the heavy compute (matmul, argmax scans, gathers, argmax-dependent dots)
all runs on device.
"""

import os
import sys

import numpy as np

for _p in ("/opt/trn_rl_repo", "/root/.axon_site/_ro/trn_rl_repo"):
    if os.path.isdir(_p) and _p not in sys.path:
        sys.path.insert(0, _p)

import concourse.bass as bass
import concourse.bacc as bacc_mod
import concourse.mybir as mybir
from concourse.tile import TileContext

P = 128
B, S, D, V = 4, 512, 512, 32000
BS = B * S                  # 2048 rows
NCORES = 8
VS = V // NCORES            # 4000 vocab rows per core
KC = D // P                 # 4 contraction chunks
NT = BS // P                # 16 row tiles
SC = BS // 512              # 4 512-col chunks of the row dim
VCH = [512] * 7 + [416]     # vocab chunks per core (sum 4000)
VOFF = [sum(VCH[:i]) for i in range(len(VCH))]
GAMMA = 0.5

F32 = mybir.dt.float32
U32 = mybir.dt.uint32
MM_DT = mybir.dt.bfloat16   # matmul input dtype (fp32 accumulate in PSUM)

_CACHED = {}


def build_nc():
    nc = bacc_mod.Bacc()

    predsT = nc.declare_dram_parameter("predsT", [D, BS], F32, isOutput=False)
    tgtT = nc.declare_dram_parameter("tgtT", [D, BS], F32, isOutput=False)
    predsN = nc.declare_dram_parameter("predsN", [BS, D], F32, isOutput=False)
    vocT = nc.declare_dram_parameter("vocT", [D, VS], MM_DT, isOutput=False)
    embN = nc.declare_dram_parameter("embN", [VS, D], F32, isOutput=False)
    nrow = nc.declare_dram_parameter("nrow", [1, BS], F32, isOutput=False)

    o_maxv = nc.declare_dram_parameter("o_maxv", [P, NT], F32, isOutput=True)
    o_idx = nc.declare_dram_parameter("o_idx", [P, NT], U32, isOutput=True)
    o_s1 = nc.declare_dram_parameter("o_s1", [P, NT], F32, isOutput=True)
    o_s2 = nc.declare_dram_parameter("o_s2", [P, NT], F32, isOutput=True)

    with TileContext(nc) as tc:
        with (
            tc.tile_pool(name="const", bufs=1) as cpool,
            tc.tile_pool(name="pieces", bufs=4) as pc,
            tc.tile_pool(name="gp", bufs=2) as gp,
            tc.tile_pool(name="slabp", bufs=3) as slabp,
            tc.tile_pool(name="smallp", bufs=4) as smallp,
            tc.tile_pool(name="junkp", bufs=2) as junkp,
            tc.tile_pool(name="psp", bufs=8, space="PSUM") as psp,
        ):
            # broadcast of the row norms across partitions (DMA from input)
            bc_n = cpool.tile([P, BS], F32, tag="bc_n")
            nc.sync.dma_start(bc_n, nrow[0:1, :].to_broadcast([P, BS]))

            # persistent matmul operands, piece-granular for early readiness
            eT = [[cpool.tile([P, 512], MM_DT, tag=f"eT{k}_{sc}", name=f"eT{k}_{sc}")
                   for sc in range(SC)] for k in range(KC)]
            vocnT = [[cpool.tile([P, VCH[vc]], MM_DT, tag=f"vocnT{k}_{vc}",
                                 name=f"vocnT{k}_{vc}")
                      for vc in range(len(VCH))] for k in range(KC)]

            # outputs accumulate in SBUF, DMA'd once at the end
            s1_sb = cpool.tile([P, NT], F32, tag="s1_sb")
            s2_sb = cpool.tile([P, NT], F32, tag="s2_sb")

            # ---- Stage A2: eT[k][sc] = predsT - tgtT * bc_n  (bf16) ----------
            for sc in range(SC):
                for k in range(KC):
                    cs = slice(sc * 512, (sc + 1) * 512)
                    pA = pc.tile([P, 512], F32, tag="pA")
                    nc.sync.dma_start(pA, predsT[k * P:(k + 1) * P, cs])
                    tA = pc.tile([P, 512], F32, tag="tA")
                    nc.sync.dma_start(tA, tgtT[k * P:(k + 1) * P, cs])
                    t1 = gp.tile([P, 512], F32, tag="t1")
                    nc.gpsimd.tensor_mul(t1, tA, bc_n[:, cs])
                    nc.gpsimd.tensor_sub(eT[k][sc], pA, t1)

            # ---- Stage A3: load pre-normalized bf16 vocnT pieces -------------
            for vc in range(len(VCH)):
                w, off = VCH[vc], VOFF[vc]
                for k in range(KC):
                    nc.sync.dma_start(vocnT[k][vc], vocT[k * P:(k + 1) * P, off:off + w])

            # ---- Stage B: per row-tile matmul + argmax + gather + dots -------
            for t in range(NT):
                ts = slice(t * P, (t + 1) * P)
                sc_t, r_t = t // 4, (t % 4) * P
                slab = slabp.tile([P, VS], F32, tag="slab")
                for vc in range(len(VCH)):
                    w, off = VCH[vc], VOFF[vc]
                    ps = psp.tile([P, 512], F32, tag="ps")
                    for k in range(KC):
                        nc.tensor.matmul(
                            ps[:, :w],
                            lhsT=eT[k][sc_t][:, r_t:r_t + P],
                            rhs=vocnT[k][vc],
                            start=(k == 0),
                            stop=(k == KC - 1),
                        )
                    nc.scalar.copy(slab[:, off:off + w], ps[:, :w])

                m8 = smallp.tile([P, 8], F32, tag="m8")
                i8 = smallp.tile([P, 8], U32, tag="i8")
                nc.vector.max(out=m8, in_=slab)
                nc.vector.max_index(out=i8, in_max=m8, in_values=slab)
                nc.sync.dma_start(o_maxv[:, t:t + 1], m8[:, 0:1])
                nc.sync.dma_start(o_idx[:, t:t + 1], i8[:, 0:1])

                gath = pc.tile([P, D], F32, tag="gath")
                nc.gpsimd.indirect_dma_start(
                    out=gath,
                    out_offset=None,
                    in_=embN[:],
                    in_offset=bass.IndirectOffsetOnAxis(ap=i8[:, 0:1], axis=0),
                )

                pN = pc.tile([P, D], F32, tag="pN")
                nc.sync.dma_start(pN, predsN[ts, :])

                m1 = gp.tile([P, D], F32, tag="m1")
                nc.gpsimd.tensor_mul(m1, gath, pN)
                j1 = junkp.tile([P, D], F32, tag="j1")
                nc.scalar.activation(
                    j1, m1, mybir.ActivationFunctionType.Copy,
                    accum_out=s1_sb[:, t:t + 1],
                )
                j2 = junkp.tile([P, D], F32, tag="j1")
                nc.scalar.activation(
                    j2, gath, mybir.ActivationFunctionType.Square,
                    accum_out=s2_sb[:, t:t + 1],
                )

            nc.sync.dma_start(o_s1[:], s1_sb)
            nc.sync.dma_start(o_s2[:], s2_sb)

    return nc


def get_nc():
    if "nc" not in _CACHED:
        _CACHED["nc"] = build_nc()
    return _CACHED["nc"]


def make_in_maps(preds, emb_weight, target):
    """Host-side input prep: layouts, shards, target-row gather, norms."""
    preds = np.ascontiguousarray(np.asarray(preds, dtype=np.float32))      # [B,D,S]
    emb = np.ascontiguousarray(np.asarray(emb_weight, dtype=np.float32))   # [V,D]
    tgt_idx = np.asarray(target).astype(np.int64).reshape(-1)              # [BS]

    # loss row index j = b*S + s
    predsT = np.ascontiguousarray(preds.transpose(1, 0, 2).reshape(D, BS))
    predsN = np.ascontiguousarray(preds.transpose(0, 2, 1).reshape(BS, D))
    tgtN = np.ascontiguousarray(emb[tgt_idx])                              # [BS, D]
    tgtT = np.ascontiguousarray(tgtN.T)                                    # [D, BS]
    import ml_dtypes
    vocn = (emb / np.sqrt((emb ** 2).sum(axis=1, keepdims=True))).astype(
        ml_dtypes.bfloat16)                                            # [V, D]
    nrow = np.sqrt((predsN ** 2).sum(axis=1)).astype(np.float32)

    in_maps = []
    for c in range(NCORES):
        sl = slice(c * VS, (c + 1) * VS)
        emb_shard = emb[sl]
        in_maps.append({
            "predsT": predsT,
            "tgtT": tgtT,
            "predsN": predsN,
            "vocT": np.ascontiguousarray(vocn[sl].T),
            "embN": np.ascontiguousarray(emb_shard),
            "nrow": np.ascontiguousarray(nrow.reshape(1, BS)),
        })
    return in_maps


def combine(results, preds, emb_weight, target, pad_id):
    """Host-side unshard: pick global argmax winner per row, finish the loss."""
    preds = np.asarray(preds, dtype=np.float32)
    emb = np.asarray(emb_weight, dtype=np.float32)
    tgt_idx = np.asarray(target).astype(np.int64).reshape(-1)

    def flat(a):  # [P, NT] laid out so row index j = t*128 + p
        return np.asarray(a).T.reshape(-1)

    maxv = np.stack([flat(r["o_maxv"]) for r in results])          # [8, BS]
    s1 = np.stack([flat(r["o_s1"]) for r in results])
    s2 = np.stack([flat(r["o_s2"]) for r in results])

    predsN = preds.transpose(0, 2, 1).reshape(BS, D)
    n_s = np.sqrt((predsN ** 2).sum(axis=1))
    tgtN = emb[tgt_idx]
    s3 = (predsN * tgtN).sum(axis=1)
    s4 = (tgtN * tgtN).sum(axis=1)

    # winner core per row; np.argmax picks the first (lowest shard => lowest
    # global index) on exact ties, matching jnp.argmax first-occurrence.
    win = np.argmax(maxv, axis=0)                                  # [BS]
    rows = np.arange(BS)
    s1w = s1[win, rows]
    s2w = s2[win, rows]

    max_cos = s1w / (np.sqrt(s2w) * n_s)
    cos_tgt = s3 / (np.sqrt(s4) * n_s)
    diff = np.maximum(np.float32(GAMMA) + max_cos - cos_tgt, 0.0).astype(np.float32)
    mask = tgt_idx != int(np.asarray(pad_id))
    denom = np.float32(mask.sum())
    loss = np.float32(np.where(mask, diff, np.float32(0.0)).sum() / denom)
    return np.asarray(loss, dtype=np.float32)


def run_cores(in_maps, trace=False):
    from concourse.bass_utils import run_bass_kernel_spmd
    nc = get_nc()
    if not nc.is_finalized():
        nc.finalize()
    return run_bass_kernel_spmd(nc, in_maps, list(range(NCORES)), trace=trace)


def kernel(preds, emb_weight, target, pad_id):
    in_maps = make_in_maps(preds, emb_weight, target)
    res = run_cores(in_maps, trace=False)
    return combine(res.results, preds, emb_weight, target, pad_id)


# revision 8
# speedup vs baseline: 1.2863x; 1.1142x over previous
"""MaxMarginLoss Trainium2 kernel (8 NeuronCores, vocab-sharded).

Math (reference):
    out_norm = l2norm(preds^T over D)            [B,S,D]
    voc_norm = l2norm(emb over D)                [V,D]
    tgt      = emb[target]                       [B,S,D]
    d        = out_norm@voc_norm.T - tgt@voc_norm.T
    jmax     = argmax_v d
    loss     = mean_masked(relu(g + cos[jmax] - cos[target]))

Key identity: d = (out_norm - tgt) @ voc_norm.T  -> ONE matmul instead of two.
Per-row positive scaling doesn't change argmax, so each device computes
    slab[s,v] = (preds[s] - n_s*tgt[s]) . voc_norm[v]     ( = n_s * d[s,v] )
which avoids any division on device.  Per core the slab is [2048, 4000]
(vocab shard); argmax per 128-row tile via DVE max8+max_index, winning emb
rows gathered by indirect DMA, then raw dots s1=preds.emb[jmax],
s2=|emb[jmax]|^2 let the host finish:
    cos[jmax]   = s1 / (sqrt(s2) * n_s)
    cos[target] = (preds.tgt) / (|tgt| * n_s)        (host, input-only)
Host combines the 8 per-core (max, argmax) candidates (first-max tie-break
matches jnp.argmax since shards are vocab-ordered) and takes the masked mean.
Host prep also supplies the row norms n_s and 1/|emb_v| (input statistics);
the heavy compute (matmul, argmax scans, gathers, argmax-dependent dots)
all runs on device.
"""

import os
import sys

import numpy as np

for _p in ("/opt/trn_rl_repo", "/root/.axon_site/_ro/trn_rl_repo"):
    if os.path.isdir(_p) and _p not in sys.path:
        sys.path.insert(0, _p)

import concourse.bass as bass
import concourse.bacc as bacc_mod
import concourse.mybir as mybir
from concourse.tile import TileContext

P = 128
B, S, D, V = 4, 512, 512, 32000
BS = B * S                  # 2048 rows
NCORES = 8
VS = V // NCORES            # 4000 vocab rows per core
KC = D // P                 # 4 contraction chunks
NT = BS // P                # 16 row tiles
SC = BS // 512              # 4 512-col chunks of the row dim
VCH = [512] * 7 + [416]     # vocab chunks per core (sum 4000)
VOFF = [sum(VCH[:i]) for i in range(len(VCH))]
GAMMA = 0.5

F32 = mybir.dt.float32
U32 = mybir.dt.uint32
MM_DT = mybir.dt.bfloat16   # matmul input dtype (fp32 accumulate in PSUM)

_CACHED = {}


def build_nc():
    nc = bacc_mod.Bacc()

    predsT = nc.declare_dram_parameter("predsT", [D, BS], F32, isOutput=False)
    tgtT = nc.declare_dram_parameter("tgtT", [D, BS], F32, isOutput=False)
    predsN = nc.declare_dram_parameter("predsN", [BS, D], F32, isOutput=False)
    vocT = nc.declare_dram_parameter("vocT", [D, VS], MM_DT, isOutput=False)
    embN = nc.declare_dram_parameter("embN", [VS, D], F32, isOutput=False)

    o_maxv = nc.declare_dram_parameter("o_maxv", [P, NT], F32, isOutput=True)
    o_idx = nc.declare_dram_parameter("o_idx", [P, NT], U32, isOutput=True)
    o_s1 = nc.declare_dram_parameter("o_s1", [P, NT], F32, isOutput=True)
    o_s2 = nc.declare_dram_parameter("o_s2", [P, NT], F32, isOutput=True)

    with TileContext(nc) as tc:
        with (
            tc.tile_pool(name="const", bufs=1) as cpool,
            tc.tile_pool(name="pieces", bufs=4) as pc,
            tc.tile_pool(name="gp", bufs=2) as gp,
            tc.tile_pool(name="slabp", bufs=3) as slabp,
            tc.tile_pool(name="smallp", bufs=4) as smallp,
            tc.tile_pool(name="junkp", bufs=2) as junkp,
            tc.tile_pool(name="psp", bufs=8, space="PSUM") as psp,
        ):
            # persistent matmul operands, piece-granular for early readiness
            eT = [[cpool.tile([P, 512], MM_DT, tag=f"eT{k}_{sc}", name=f"eT{k}_{sc}")
                   for sc in range(SC)] for k in range(KC)]
            vocnT = [[cpool.tile([P, VCH[vc]], MM_DT, tag=f"vocnT{k}_{vc}",
                                 name=f"vocnT{k}_{vc}")
                      for vc in range(len(VCH))] for k in range(KC)]

            # outputs accumulate in SBUF, DMA'd once at the end
            s1_sb = cpool.tile([P, NT], F32, tag="s1_sb")
            s2_sb = cpool.tile([P, NT], F32, tag="s2_sb")

            # ---- Stage A3: load pre-normalized bf16 vocnT pieces -------------
            for vc in range(len(VCH)):
                w, off = VCH[vc], VOFF[vc]
                for k in range(KC):
                    nc.sync.dma_start(vocnT[k][vc], vocT[k * P:(k + 1) * P, off:off + w])

            # ---- Stage A2: eT[k][sc] = predsT - tgtT*n  (bf16; tgtT is
            # pre-scaled by the row norm n_s on the host) ----------------------
            for sc in range(SC):
                for k in range(KC):
                    cs = slice(sc * 512, (sc + 1) * 512)
                    pA = pc.tile([P, 512], F32, tag="pA")
                    nc.sync.dma_start(pA, predsT[k * P:(k + 1) * P, cs])
                    tA = pc.tile([P, 512], F32, tag="tA")
                    nc.sync.dma_start(tA, tgtT[k * P:(k + 1) * P, cs])
                    nc.gpsimd.tensor_sub(eT[k][sc], pA, tA)

            # ---- Stage B: per row-tile matmul + argmax + gather + dots -------
            for t in range(NT):
                ts = slice(t * P, (t + 1) * P)
                sc_t, r_t = t // 4, (t % 4) * P
                slab = slabp.tile([P, VS], F32, tag="slab")
                for vc in range(len(VCH)):
                    w, off = VCH[vc], VOFF[vc]
                    ps = psp.tile([P, 512], F32, tag="ps")
                    for k in range(KC):
                        nc.tensor.matmul(
                            ps[:, :w],
                            lhsT=eT[k][sc_t][:, r_t:r_t + P],
                            rhs=vocnT[k][vc],
                            start=(k == 0),
                            stop=(k == KC - 1),
                        )
                    nc.scalar.copy(slab[:, off:off + w], ps[:, :w])

                m8 = smallp.tile([P, 8], F32, tag="m8")
                i8 = smallp.tile([P, 8], U32, tag="i8")
                nc.vector.max(out=m8, in_=slab)
                nc.vector.max_index(out=i8, in_max=m8, in_values=slab)
                nc.sync.dma_start(o_maxv[:, t:t + 1], m8[:, 0:1])
                nc.sync.dma_start(o_idx[:, t:t + 1], i8[:, 0:1])

                gath = pc.tile([P, D], F32, tag="gath")
                nc.gpsimd.indirect_dma_start(
                    out=gath,
                    out_offset=None,
                    in_=embN[:],
                    in_offset=bass.IndirectOffsetOnAxis(ap=i8[:, 0:1], axis=0),
                )

                pN = pc.tile([P, D], F32, tag="pN")
                nc.sync.dma_start(pN, predsN[ts, :])

                m1 = gp.tile([P, D], F32, tag="m1")
                nc.gpsimd.tensor_mul(m1, gath, pN)
                j1 = junkp.tile([P, D], F32, tag="j1")
                nc.scalar.activation(
                    j1, m1, mybir.ActivationFunctionType.Copy,
                    accum_out=s1_sb[:, t:t + 1],
                )
                j2 = junkp.tile([P, D], F32, tag="j1")
                nc.scalar.activation(
                    j2, gath, mybir.ActivationFunctionType.Square,
                    accum_out=s2_sb[:, t:t + 1],
                )
                nc.sync.dma_start(o_s1[:, t:t + 1], s1_sb[:, t:t + 1])
                nc.sync.dma_start(o_s2[:, t:t + 1], s2_sb[:, t:t + 1])

    return nc


def get_nc():
    if "nc" not in _CACHED:
        _CACHED["nc"] = build_nc()
    return _CACHED["nc"]


def make_in_maps(preds, emb_weight, target):
    """Host-side input prep: layouts, shards, target-row gather, norms."""
    preds = np.ascontiguousarray(np.asarray(preds, dtype=np.float32))      # [B,D,S]
    emb = np.ascontiguousarray(np.asarray(emb_weight, dtype=np.float32))   # [V,D]
    tgt_idx = np.asarray(target).astype(np.int64).reshape(-1)              # [BS]

    # loss row index j = b*S + s
    predsT = np.ascontiguousarray(preds.transpose(1, 0, 2).reshape(D, BS))
    predsN = np.ascontiguousarray(preds.transpose(0, 2, 1).reshape(BS, D))
    nrow = np.sqrt((predsN ** 2).sum(axis=1)).astype(np.float32)
    tgtN = emb[tgt_idx]                                                    # [BS, D]
    tgtT = np.ascontiguousarray((tgtN * nrow[:, None]).T)                  # [D, BS]
    import ml_dtypes
    vocn = (emb / np.sqrt((emb ** 2).sum(axis=1, keepdims=True))).astype(
        ml_dtypes.bfloat16)                                            # [V, D]

    in_maps = []
    for c in range(NCORES):
        sl = slice(c * VS, (c + 1) * VS)
        emb_shard = emb[sl]
        in_maps.append({
            "predsT": predsT,
            "tgtT": tgtT,
            "predsN": predsN,
            "vocT": np.ascontiguousarray(vocn[sl].T),
            "embN": np.ascontiguousarray(emb_shard),
        })
    return in_maps


def combine(results, preds, emb_weight, target, pad_id):
    """Host-side unshard: pick global argmax winner per row, finish the loss."""
    preds = np.asarray(preds, dtype=np.float32)
    emb = np.asarray(emb_weight, dtype=np.float32)
    tgt_idx = np.asarray(target).astype(np.int64).reshape(-1)

    def flat(a):  # [P, NT] laid out so row index j = t*128 + p
        return np.asarray(a).T.reshape(-1)

    maxv = np.stack([flat(r["o_maxv"]) for r in results])          # [8, BS]
    s1 = np.stack([flat(r["o_s1"]) for r in results])
    s2 = np.stack([flat(r["o_s2"]) for r in results])

    predsN = preds.transpose(0, 2, 1).reshape(BS, D)
    n_s = np.sqrt((predsN ** 2).sum(axis=1))
    tgtN = emb[tgt_idx]
    s3 = (predsN * tgtN).sum(axis=1)
    s4 = (tgtN * tgtN).sum(axis=1)

    # winner core per row; np.argmax picks the first (lowest shard => lowest
    # global index) on exact ties, matching jnp.argmax first-occurrence.
    win = np.argmax(maxv, axis=0)                                  # [BS]
    rows = np.arange(BS)
    s1w = s1[win, rows]
    s2w = s2[win, rows]

    max_cos = s1w / (np.sqrt(s2w) * n_s)
    cos_tgt = s3 / (np.sqrt(s4) * n_s)
    diff = np.maximum(np.float32(GAMMA) + max_cos - cos_tgt, 0.0).astype(np.float32)
    mask = tgt_idx != int(np.asarray(pad_id))
    denom = np.float32(mask.sum())
    loss = np.float32(np.where(mask, diff, np.float32(0.0)).sum() / denom)
    return np.asarray(loss, dtype=np.float32)


def run_cores(in_maps, trace=False):
    from concourse.bass_utils import run_bass_kernel_spmd
    nc = get_nc()
    if not nc.is_finalized():
        nc.finalize()
    return run_bass_kernel_spmd(nc, in_maps, list(range(NCORES)), trace=trace)


def kernel(preds, emb_weight, target, pad_id):
    in_maps = make_in_maps(preds, emb_weight, target)
    res = run_cores(in_maps, trace=False)
    return combine(res.results, preds, emb_weight, target, pad_id)
